# revision 11
# baseline (speedup 1.0000x reference)
"""Trainium2 Bass kernel for BasicAttention.

reference math (fp32):
  xf = x.reshape(b, din, hw)               # b=4, din=256, hw=4096
  Q = q_w @ xf   [b, 64, hw]
  K = k_w @ xf   [b, 64, hw]
  V = v_w @ xf   [b, 256, hw]
  S = Q^T K      [b, hw, hw]
  A = softmax(S, axis=-1)
  z = (A @ V^T)^T -> [b, 256, h, w]

Sharding: 8 cores = (batch b in 0..4) x (query half in 0..2). Each core gets
its batch's full xf with columns rotated so its 2048 queries come first
(attention is permutation-invariant over keys, so K/V built from the rotated
xf give identical outputs).

Per-core dataflow (fp32 PSUM, 16-bit matmuls):
  - Weights arrive HOST-PACKED (wkq [128, 256] = [k_h0|q_h0|k_h1|q_h1],
    wv [128, 512] = [v_h0|v_h1], fp16) so the weight DMAs are contiguous
    512B/1KB-row transfers instead of 128-byte strided gathers; k_w is
    pre-halved on the host for the K=128 duplication trick below.
  - k2/q2 [128, *] hold K/2 and Q duplicated on both partition halves, so S
    matmuls contract over K=128: S = (K/2)^T Q + (K/2)^T Q.  K=64 matmuls
    stream the same 512 rows but half the array idles; K=128 gets full use.
  - S psums are [128, 1024] pairs (2 banks); ONE ACT exp per pair reads
    across both banks, halving ACT instruction count.
  - Z accumulates vt^T @ exp into two [128,512] psums.
  - Softmax denominators: DVE accumulates exp pairs on two interleaved
    chains (Pool takes pairs 1,3,5,7,9); the remaining pairs are summed
    directly on PE with ones[128,128] bf16 matmuls into a sums psum, plus
    one bf16 matmul over the folded chain accumulator (accf, bf16).
  - Non-last query tiles: pairs 12-15 on PE, deferred into the next tile's
    stream (evict psum at i==0, denominator+normalize+store at i==2).
  - LAST query tile: pairs 11-15 on PE, their sums matmuls interleaved
    right after each pair's Z matmuls, accf matmul last; normalize
    multiplies read the Z psums directly (no SBUF bounce) and the final
    output DMA is split across 4 engine rings.  Tail after the last Z
    matmul ~3us instead of ~7.5us.
  - Output is fp16 (host upcasts); halves output DMA bytes.
"""

import sys
import os

sys.path.insert(0, "/opt/trn_rl_repo")

import numpy as np

B, DIN, H, W = 4, 256, 64, 64
HW = H * W            # 4096 keys
DK, DV = 64, 256
PQ = HW // 2          # 2048 queries per core
PT = 512              # query tile (psum free dim)
QC = 128              # key chunk (contraction tile)
NPT = PQ // PT        # 4
NQC = HW // QC        # 32
PAIRS = NQC // 2      # 16
POOL_PAIRS = (1, 3, 5, 7, 9)     # pair-adds done on GpSimd (all pts)
PE_PAIRS = (12, 13, 14, 15)      # non-last pts: summed on PE, deferred
PE_PAIRS_LAST = (11, 12, 13, 14, 15)  # last pt: summed on PE, interleaved
N_CORES = 8

_cache = {}


def _build():
    if "nc" in _cache:
        return _cache["nc"]

    from contextlib import ExitStack
    import concourse.tile as tile
    from concourse import bacc, mybir

    f32 = mybir.dt.float32
    bf16 = mybir.dt.bfloat16
    fp16 = mybir.dt.float16

    nc = bacc.Bacc("TRN2", target_bir_lowering=False, debug=False,
                   num_devices=N_CORES)

    xb = nc.dram_tensor("xb", [DIN, HW], fp16, kind="ExternalInput").ap()
    wkq_d = nc.dram_tensor("wkq", [128, 4 * DK], fp16,
                           kind="ExternalInput").ap()
    wv_d = nc.dram_tensor("wv", [128, 2 * DV], fp16,
                          kind="ExternalInput").ap()
    zout = nc.dram_tensor("zout", [DV, PQ], fp16, kind="ExternalOutput").ap()

    with tile.TileContext(nc) as tc, ExitStack() as ctx:
        singles = ctx.enter_context(tc.tile_pool(name="singles", bufs=1))
        vt_pool = ctx.enter_context(tc.tile_pool(name="vt_pool", bufs=NQC))
        exps_pool = ctx.enter_context(tc.tile_pool(name="exps_pool", bufs=8))
        sum_pool = ctx.enter_context(tc.tile_pool(name="sum_pool", bufs=2))
        out_pool = ctx.enter_context(tc.tile_pool(name="out_pool", bufs=2))
        ps_s = ctx.enter_context(tc.tile_pool(name="ps_s", bufs=2,
                                              space="PSUM"))
        ps_z = ctx.enter_context(tc.tile_pool(name="ps_z", bufs=1,
                                              space="PSUM"))
        ps_w = ctx.enter_context(tc.tile_pool(name="ps_w", bufs=2,
                                              space="PSUM"))

        w_kq = singles.tile([128, 4 * DK], fp16)
        w_v = singles.tile([128, 2 * DV], fp16)

        k2 = singles.tile([128, HW], fp16)    # K/2 on both partition halves
        q2 = singles.tile([128, PQ], fp16)    # Q on both partition halves
        xf0 = singles.tile([128, HW], fp16)
        xf1 = singles.tile([128, HW], fp16)

        # ---- DMA cost is ~18ns per partition ROW + bytes: few fat pieces,
        # weights on the gpsimd ring so xf streams unimpeded on sync/scalar
        pieces = [(0, 1024), (1024, 2048), (2048, 3072), (3072, 4096)]
        nc.gpsimd.dma_start(out=w_kq, in_=wkq_d)
        nc.gpsimd.dma_start(out=w_v, in_=wv_d)
        for lo, hi in pieces:
            nc.sync.dma_start(out=xf0[:, lo:hi], in_=xb[0:128, lo:hi])
            nc.scalar.dma_start(out=xf1[:, lo:hi], in_=xb[128:256, lo:hi])

        ones_b = singles.tile([128, 128], bf16)
        nc.gpsimd.memset(ones_b, 1.0)

        vt = [None] * NQC

        def proj_kq(g):
            """K/Q projections for x cols [g*1024, (g+1)*1024)."""
            for j in range(2 * g, 2 * g + 2):     # 512-col groups
                sl = slice(j * PT, (j + 1) * PT)
                if j < PQ // PT:
                    # fused K+Q projection: psum rows 0:64=K/2, 64:128=Q
                    pk = ps_w.tile([128, PT], f32, name=f"pk{j}", tag="scr")
                    nc.tensor.matmul(pk, w_kq[:, 0:2 * DK], xf0[:, sl],
                                     start=True, stop=False)
                    nc.tensor.matmul(pk, w_kq[:, 2 * DK:4 * DK], xf1[:, sl],
                                     start=False, stop=True)
                    nc.vector.tensor_copy(k2[0:64, sl], pk[0:64, :])
                    nc.scalar.copy(k2[64:128, sl], pk[0:64, :])
                    nc.vector.tensor_copy(q2[0:64, sl], pk[64:128, :])
                    nc.scalar.copy(q2[64:128, sl], pk[64:128, :])
                else:
                    pk = ps_w.tile([64, PT], f32, name=f"pk{j}", tag="scr")
                    nc.tensor.matmul(pk, w_kq[:, 0:DK], xf0[:, sl],
                                     start=True, stop=False)
                    nc.tensor.matmul(pk, w_kq[:, 2 * DK:3 * DK], xf1[:, sl],
                                     start=False, stop=True)
                    nc.vector.tensor_copy(k2[0:64, sl], pk)
                    nc.scalar.copy(k2[64:128, sl], pk)

        def proj_v(g):
            """V projections for x cols [g*1024, (g+1)*1024)."""
            for qc in range(8 * g, 8 * g + 8):
                sl = slice(qc * QC, (qc + 1) * QC)
                pv = ps_w.tile([QC, DV], f32, name=f"pv{qc}", tag="scr")
                nc.tensor.matmul(pv, xf0[:, sl], w_v[:, 0:DV],
                                 start=True, stop=False)
                nc.tensor.matmul(pv, xf1[:, sl], w_v[:, DV:2 * DV],
                                 start=False, stop=True)
                vt_t = vt_pool.tile([QC, DV], bf16, name=f"vt{qc}", tag="vt")
                if qc % 2 == 0:
                    nc.vector.tensor_copy(vt_t, pv)
                else:
                    nc.scalar.copy(vt_t, pv)
                vt[qc] = vt_t

        def proj_chunk(g):
            proj_kq(g)
            proj_v(g)

        # ---- attention main loop over query tiles, in chunk PAIRS ----
        EXP = mybir.ActivationFunctionType.Exp

        def s_pair(pt, i):
            qs = q2[:, pt * PT:(pt + 1) * PT]
            t = ps_s.tile([128, 2 * PT], f32, name=f"sp{pt}_{i}",
                          tag="spair")
            nc.tensor.matmul(t[:, 0:PT], k2[:, (2 * i) * QC:
                                             (2 * i + 1) * QC],
                             qs, start=True, stop=True)
            nc.tensor.matmul(t[:, PT:2 * PT], k2[:, (2 * i + 1) * QC:
                                                 (2 * i + 2) * QC],
                             qs, start=True, stop=True)
            return t

        def mk_exp(pt, i, t):
            e = exps_pool.tile([128, 2 * PT], bf16, name=f"e{pt}_{i}",
                               tag="exps")
            nc.scalar.activation(e, t, func=EXP)
            return e

        def preheat(pt):
            """Pair 0 of ptile pt via the scratch psum banks + pair 1's S,
            emitted while the previous ptile is still streaming, so the
            next ptile's Z matmuls can start with zero PE idle."""
            qs = q2[:, pt * PT:(pt + 1) * PT]
            sa = ps_w.tile([128, PT], f32, name=f"sa{pt}", tag="scr")
            nc.tensor.matmul(sa, k2[:, 0:QC], qs, start=True, stop=True)
            sb = ps_w.tile([128, PT], f32, name=f"sb{pt}", tag="scr")
            nc.tensor.matmul(sb, k2[:, QC:2 * QC], qs, start=True,
                             stop=True)
            s1 = s_pair(pt, 1)
            e0 = exps_pool.tile([128, 2 * PT], bf16, name=f"e{pt}_0",
                                tag="exps")
            nc.scalar.activation(e0[:, 0:PT], sa, func=EXP)
            nc.scalar.activation(e0[:, PT:2 * PT], sb, func=EXP)
            return e0, s1

        def emit_sums(pt, accf, saved_e):
            sums = ps_w.tile([128, PT], f32, name=f"sums{pt}", tag="scr")
            nc.tensor.matmul(sums, ones_b, accf, start=True, stop=False)
            for idx, j in enumerate(PE_PAIRS):
                e = saved_e[j]
                nc.tensor.matmul(sums, ones_b, e[:, 0:PT],
                                 start=False, stop=False)
                nc.tensor.matmul(sums, ones_b, e[:, PT:2 * PT],
                                 start=False, stop=(idx == len(PE_PAIRS) - 1))
            return sums

        tail_finish = None
        proj_kq(0)
        ph = preheat(0)
        proj_v(0)
        for pt in range(NPT):
            last = pt == NPT - 1
            pz0 = ps_z.tile([128, PT], f32, name=f"pz0_{pt}", tag="pz0")
            pz1 = ps_z.tile([128, PT], f32, name=f"pz1_{pt}", tag="pz1")
            acc_a = sum_pool.tile([128, 2 * PT], f32, name=f"acca{pt}",
                                  tag="acca")
            acc_b = sum_pool.tile([128, 2 * PT], f32, name=f"accb{pt}",
                                  tag="accb")
            acc_p = sum_pool.tile([128, 2 * PT], f32, name=f"accp{pt}",
                                  tag="accp")
            saved_e = {}
            ebuf = {}
            sums_ps = None

            pend = [ph[1]]
            E = {0: ph[0]}

            for i in range(PAIRS):
                if i == 0 and tail_finish is not None:
                    tail_finish[0]()  # evict prev pz before Z reuses banks
                if i == 2 and tail_finish is not None:
                    tail_finish[1]()  # prev denominator + normalize + store
                    tail_finish = None
                # Z first: never let an S matmul waiting on an xf piece (or
                # a proj matmul) block ready Z work in the in-order PE queue
                e = E.pop(i)
                c0, c1 = 2 * i, 2 * i + 1
                e0, e1 = e[:, 0:PT], e[:, PT:2 * PT]
                nc.tensor.matmul(pz0, vt[c0][:, 0:128], e0,
                                 start=(i == 0), stop=False)
                nc.tensor.matmul(pz0, vt[c1][:, 0:128], e1,
                                 start=False, stop=(i == PAIRS - 1))
                nc.tensor.matmul(pz1, vt[c0][:, 128:256], e0,
                                 start=(i == 0), stop=False)
                nc.tensor.matmul(pz1, vt[c1][:, 128:256], e1,
                                 start=False, stop=(i == PAIRS - 1))
                if pt == 0 and i in (2, 5, 9):
                    proj_chunk({2: 1, 5: 2, 9: 3}[i])
                if i + 2 < PAIRS:
                    pend.append(s_pair(pt, i + 2))
                if i + 1 < PAIRS:
                    E[i + 1] = mk_exp(pt, i + 1, pend.pop(0))
                ebuf[i] = e
                if (not last) and i in PE_PAIRS:
                    saved_e[i] = e       # summed on PE after the last Z
                if last and i in PE_PAIRS_LAST:
                    # interleave denominator matmuls with the Z stream
                    if sums_ps is None:
                        sums_ps = ps_w.tile([128, PT], f32,
                                            name=f"sums{pt}", tag="scr")
                        nc.tensor.matmul(sums_ps, ones_b, e0,
                                         start=True, stop=False)
                    else:
                        nc.tensor.matmul(sums_ps, ones_b, e0,
                                         start=False, stop=False)
                    nc.tensor.matmul(sums_ps, ones_b, e1,
                                     start=False, stop=False)
                # Pool chain: pairs 1,3,5,7,9; two-operand first add
                if i == 3:
                    nc.gpsimd.tensor_add(acc_p, ebuf[1], ebuf[3])
                elif i in (5, 7, 9):
                    nc.gpsimd.tensor_add(acc_p, acc_p, ebuf[i])
                if i == 10:
                    # Pool folds its own accumulator while DVE still adds
                    acc_pr = sum_pool.tile([128, PT], f32, name=f"apr{pt}",
                                           tag="accpr")
                    nc.gpsimd.tensor_add(acc_pr, acc_p[:, 0:PT],
                                         acc_p[:, PT:2 * PT])
                # DVE chains: non-last a={0,4,8,11} b={2,6,10};
                #             last     a={0,4,8}    b={2,6,10}
                if i == 4:
                    nc.vector.tensor_add(acc_a, ebuf[0], ebuf[4])
                elif i == 6:
                    nc.vector.tensor_add(acc_b, ebuf[2], ebuf[6])
                elif i == 8:
                    nc.vector.tensor_add(acc_a, acc_a, ebuf[8])
                elif i == 10:
                    nc.vector.tensor_add(acc_b, acc_b, ebuf[10])
                elif i == 11 and not last:
                    nc.vector.tensor_add(acc_a, acc_a, ebuf[11])
                if i == (10 if last else 11):
                    # fold-first merge: four short [128, PT] adds
                    acc_ua = sum_pool.tile([128, PT], f32, name=f"ua{pt}",
                                           tag="accua")
                    nc.vector.tensor_add(acc_ua, acc_a[:, 0:PT],
                                         acc_a[:, PT:2 * PT])
                    acc_ub = sum_pool.tile([128, PT], f32, name=f"ub{pt}",
                                           tag="accub")
                    nc.vector.tensor_add(acc_ub, acc_b[:, 0:PT],
                                         acc_b[:, PT:2 * PT])
                    acc_u = sum_pool.tile([128, PT], f32, name=f"au{pt}",
                                          tag="accu")
                    nc.vector.tensor_add(acc_u, acc_ua, acc_ub)
                if i == 11:
                    accf = sum_pool.tile([128, PT], bf16, name=f"af{pt}",
                                         tag="accf")
                    nc.vector.tensor_add(accf, acc_u, acc_pr)
                if i == 14 and pt + 1 < NPT:
                    ph = preheat(pt + 1)

            if last:
                # accf matmul last (chains have finished by now), then the
                # short normalize+store tail straight out of PSUM
                nc.tensor.matmul(sums_ps, ones_b, accf,
                                 start=False, stop=True)
                bcast = sum_pool.tile([128, PT], f32, name=f"bc{pt}",
                                      tag="bcast")
                nc.vector.reciprocal_approx_fast(out=bcast, in_=sums_ps)
                out0 = out_pool.tile([128, PT], fp16, name=f"o0_{pt}",
                                     tag="out0")
                out1 = out_pool.tile([128, PT], fp16, name=f"o1_{pt}",
                                     tag="out1")
                nc.vector.tensor_mul(out0, pz0, bcast)
                nc.vector.tensor_mul(out1, pz1, bcast)
                # split by partition rows (DMA cost ~18ns/row) over 3 rings
                base = pt * PT
                sl = slice(base, base + PT)
                rows = [(0, 43), (43, 86), (86, 128)]
                engs = [nc.sync, nc.scalar, nc.gpsimd]
                for (r0, r1), eng in zip(rows, engs):
                    eng.dma_start(out=zout[r0:r1, sl], in_=out0[r0:r1, :])
                for (r0, r1), eng in zip(rows, engs):
                    eng.dma_start(out=zout[128 + r0:128 + r1, sl],
                                  in_=out1[r0:r1, :])
            else:
                # ---- deferred tail: evict pz, then denominator+normalize
                def make_tail(pt=pt, pz0=pz0, pz1=pz1, accf=accf,
                              saved_e=dict(saved_e)):
                    st = {}

                    def evict():
                        st["zr0"] = out_pool.tile([128, PT], f32,
                                                  name=f"zr0_{pt}",
                                                  tag="zr0")
                        st["zr1"] = out_pool.tile([128, PT], f32,
                                                  name=f"zr1_{pt}",
                                                  tag="zr1")
                        nc.vector.tensor_copy(st["zr0"], pz0)
                        nc.vector.tensor_copy(st["zr1"], pz1)

                    def finish():
                        sums = emit_sums(pt, accf, saved_e)
                        bcast = sum_pool.tile([128, PT], f32, name=f"bc{pt}",
                                              tag="bcast")
                        nc.vector.reciprocal_approx_fast(out=bcast, in_=sums)
                        out0 = out_pool.tile([128, PT], fp16,
                                             name=f"o0_{pt}", tag="out0")
                        out1 = out_pool.tile([128, PT], fp16,
                                             name=f"o1_{pt}", tag="out1")
                        nc.vector.tensor_mul(out0, st["zr0"], bcast)
                        nc.vector.tensor_mul(out1, st["zr1"], bcast)
                        nc.sync.dma_start(
                            out=zout[0:128, pt * PT:(pt + 1) * PT], in_=out0)
                        nc.sync.dma_start(
                            out=zout[128:256, pt * PT:(pt + 1) * PT],
                            in_=out1)
                    return evict, finish

                tail_finish = make_tail()

    nc.compile()
    _cache["nc"] = nc
    return nc


def _in_maps(x, q_w, k_w, v_w):
    xf = np.ascontiguousarray(x.reshape(B, DIN, HW), dtype=np.float32)
    qwT = np.asarray(q_w, np.float32).T            # [din, dk]
    # k_w halved: k2 holds K/2 on both partition halves, S contracts K=128
    kwT = np.asarray(k_w, np.float32).T * 0.5
    vwT = np.asarray(v_w, np.float32).T            # [din, dv]
    wkq = np.concatenate(
        [kwT[0:128], qwT[0:128], kwT[128:256], qwT[128:256]],
        axis=1).astype(np.float16)                 # [128, 4*dk]
    wv = np.concatenate([vwT[0:128], vwT[128:256]],
                        axis=1).astype(np.float16)  # [128, 2*dv]
    maps = []
    for c in range(N_CORES):
        b, half = divmod(c, 2)
        xbc = xf[b] if half == 0 else np.ascontiguousarray(
            np.roll(xf[b], -PQ, axis=1))
        maps.append({"xb": xbc.astype(np.float16), "wkq": wkq, "wv": wv})
    return maps


def _gather(results):
    z = np.empty((B, DV, HW), np.float32)
    for c in range(N_CORES):
        b, half = divmod(c, 2)
        z[b][:, half * PQ:(half + 1) * PQ] = \
            results[c]["zout"].astype(np.float32)
    return z.reshape(B, DV, H, W)


def _run(x, q_w, k_w, v_w, trace=False):
    from concourse import bass_utils
    nc = _build()
    res = bass_utils.run_bass_kernel_spmd(
        nc, _in_maps(x, q_w, k_w, v_w), core_ids=list(range(N_CORES)),
        trace=trace)
    return _gather(res.results), res


def kernel(x, q_w, k_w, v_w):
    z, _ = _run(x, q_w, k_w, v_w)
    return z


# revision 12
# speedup vs baseline: 1.2776x; 1.2776x over previous
"""Trainium2 Bass kernel for BasicAttention.

reference math (fp32):
  xf = x.reshape(b, din, hw)               # b=4, din=256, hw=4096
  Q = q_w @ xf   [b, 64, hw]
  K = k_w @ xf   [b, 64, hw]
  V = v_w @ xf   [b, 256, hw]
  S = Q^T K      [b, hw, hw]
  A = softmax(S, axis=-1)
  z = (A @ V^T)^T -> [b, 256, h, w]

Sharding: 8 cores = (batch b in 0..4) x (query half in 0..2). Each core gets
its batch's full xf with columns rotated so its 2048 queries come first
(attention is permutation-invariant over keys, so K/V built from the rotated
xf give identical outputs).

Input layout: DMA costs ~18ns per PARTITION ROW + bytes, so weights are
EMBEDDED in the x tensor (one [256, 384+4096] fp16 input): row-half r of
xb is [wkq_hr (128) | wv_hr (256) | x rows of half r (4096)].  The first
1408-col piece per half then delivers weights + the first 1024 x columns
in a single transfer per ring; no separate weight DMAs, nothing on the
slow gpsimd ring at startup.  k_w is pre-halved on the host for the K=128
duplication trick below.

Per-core dataflow (fp32 PSUM, 16-bit matmuls):
  - k2/q2 [128, *] hold K/2 and Q duplicated on both partition halves, so S
    matmuls contract over K=128: S = (K/2)^T Q + (K/2)^T Q.
  - S psums are [128, 1024] pairs (2 banks); ONE ACT exp per pair.
  - Z accumulates vt^T @ exp into two [128,512] psums.
  - Emission order per slot keeps S-pair prefetch (i+2) AHEAD of Z(i): the
    PE pipeline must never see a dependency bubble — any idle gap drops
    the Tensor engine out of its ~3us p-state ramp and the whole stream
    runs ~20% slower (measured 216ns -> 259ns per 512-row matmul).
  - Softmax denominators: DVE chains a={0,4,8(,11)} b={2,6,10}, Pool
    {1,3,5,7,9}, folded early ([128,512] adds) into a bf16 accf; the
    remaining pairs are summed on PE with ones bf16 matmuls.
  - Non-last ptiles: pairs 12-15 on PE, deferred into the next tile
    (evict psum at i==0, denominator+normalize+store at i==2).
  - LAST ptile: pairs 11-15 on PE, sums matmuls interleaved right after
    each pair's Z matmuls, accf matmul last; normalize reads the Z psums
    directly and the output DMA is split across the 3 rings.
  - Output is fp16 (host upcasts); halves output DMA bytes.
"""

import sys
import os

sys.path.insert(0, "/opt/trn_rl_repo")

import numpy as np

B, DIN, H, W = 4, 256, 64, 64
HW = H * W            # 4096 keys
DK, DV = 64, 256
PQ = HW // 2          # 2048 queries per core
PT = 512              # query tile (psum free dim)
QC = 128              # key chunk (contraction tile)
NPT = PQ // PT        # 4
NQC = HW // QC        # 32
PAIRS = NQC // 2      # 16
WCOL = 2 * DK + DV    # 384 embedded weight columns per half
XW = WCOL + HW        # 4480 total input columns
POOL_PAIRS = (1, 3, 5, 7, 9)
PE_PAIRS = (12, 13, 14, 15)           # non-last ptiles, deferred
PE_PAIRS_LAST = (11, 12, 13, 14, 15)  # last ptile, interleaved
N_CORES = 8

_cache = {}


def _build():
    if "nc" in _cache:
        return _cache["nc"]

    from contextlib import ExitStack
    import concourse.tile as tile
    from concourse import bacc, mybir

    f32 = mybir.dt.float32
    bf16 = mybir.dt.bfloat16
    fp16 = mybir.dt.float16

    nc = bacc.Bacc("TRN2", target_bir_lowering=False, debug=False,
                   num_devices=N_CORES)

    xb = nc.dram_tensor("xb", [DIN, XW], fp16, kind="ExternalInput").ap()
    zout = nc.dram_tensor("zout", [DV, PQ], fp16, kind="ExternalOutput").ap()

    with tile.TileContext(nc) as tc, ExitStack() as ctx:
        singles = ctx.enter_context(tc.tile_pool(name="singles", bufs=1))
        vt_pool = ctx.enter_context(tc.tile_pool(name="vt_pool", bufs=NQC))
        exps_pool = ctx.enter_context(tc.tile_pool(name="exps_pool", bufs=8))
        sum_pool = ctx.enter_context(tc.tile_pool(name="sum_pool", bufs=2))
        out_pool = ctx.enter_context(tc.tile_pool(name="out_pool", bufs=2))
        ps_s = ctx.enter_context(tc.tile_pool(name="ps_s", bufs=2,
                                              space="PSUM"))
        ps_z = ctx.enter_context(tc.tile_pool(name="ps_z", bufs=1,
                                              space="PSUM"))
        ps_w = ctx.enter_context(tc.tile_pool(name="ps_w", bufs=2,
                                              space="PSUM"))

        k2 = singles.tile([128, HW], fp16)    # K/2 on both partition halves
        q2 = singles.tile([128, PQ], fp16)    # Q on both partition halves
        xf0 = singles.tile([128, XW], fp16)   # [wkq_h0 | wv_h0 | x half 0]
        xf1 = singles.tile([128, XW], fp16)   # [wkq_h1 | wv_h1 | x half 1]

        # weight views into the embedded columns
        wkqa = xf0[:, 0:2 * DK]          # [k_h0 | q_h0]
        wkqb = xf1[:, 0:2 * DK]          # [k_h1 | q_h1]
        wva = xf0[:, 2 * DK:WCOL]
        wvb = xf1[:, 2 * DK:WCOL]

        # 4 fat pieces per half; piece 1 carries weights + first 1024 x cols
        pieces = [(0, WCOL + 1024), (WCOL + 1024, WCOL + 2048),
                  (WCOL + 2048, WCOL + 3072), (WCOL + 3072, XW)]
        for lo, hi in pieces:
            nc.sync.dma_start(out=xf0[:, lo:hi], in_=xb[0:128, lo:hi])
            nc.scalar.dma_start(out=xf1[:, lo:hi], in_=xb[128:256, lo:hi])

        ones_b = singles.tile([128, 128], bf16)
        nc.gpsimd.memset(ones_b, 1.0)

        vt = [None] * NQC

        def xsl(lo, hi):
            return slice(WCOL + lo, WCOL + hi)

        def proj_kq(g):
            """K/Q projections for x cols [g*1024, (g+1)*1024)."""
            for j in range(2 * g, 2 * g + 2):     # 512-col groups
                sl = slice(j * PT, (j + 1) * PT)
                xs = xsl(j * PT, (j + 1) * PT)
                if j < PQ // PT:
                    # fused K+Q projection: psum rows 0:64=K/2, 64:128=Q
                    pk = ps_w.tile([128, PT], f32, name=f"pk{j}", tag="scr")
                    nc.tensor.matmul(pk, wkqa, xf0[:, xs],
                                     start=True, stop=False)
                    nc.tensor.matmul(pk, wkqb, xf1[:, xs],
                                     start=False, stop=True)
                    nc.vector.tensor_copy(k2[0:64, sl], pk[0:64, :])
                    nc.scalar.copy(k2[64:128, sl], pk[0:64, :])
                    nc.vector.tensor_copy(q2[0:64, sl], pk[64:128, :])
                    nc.scalar.copy(q2[64:128, sl], pk[64:128, :])
                else:
                    pk = ps_w.tile([64, PT], f32, name=f"pk{j}", tag="scr")
                    nc.tensor.matmul(pk, wkqa[:, 0:DK], xf0[:, xs],
                                     start=True, stop=False)
                    nc.tensor.matmul(pk, wkqb[:, 0:DK], xf1[:, xs],
                                     start=False, stop=True)
                    nc.vector.tensor_copy(k2[0:64, sl], pk)
                    nc.scalar.copy(k2[64:128, sl], pk)

        def proj_v(g):
            """V projections for x cols [g*1024, (g+1)*1024)."""
            for qc in range(8 * g, 8 * g + 8):
                xs = xsl(qc * QC, (qc + 1) * QC)
                pv = ps_w.tile([QC, DV], f32, name=f"pv{qc}", tag="scr")
                nc.tensor.matmul(pv, xf0[:, xs], wva,
                                 start=True, stop=False)
                nc.tensor.matmul(pv, xf1[:, xs], wvb,
                                 start=False, stop=True)
                vt_t = vt_pool.tile([QC, DV], bf16, name=f"vt{qc}", tag="vt")
                if qc % 2 == 0:
                    nc.vector.tensor_copy(vt_t, pv)
                else:
                    nc.scalar.copy(vt_t, pv)
                vt[qc] = vt_t

        def proj_chunk(g):
            proj_kq(g)
            proj_v(g)

        # ---- attention main loop over query tiles, in chunk PAIRS ----
        EXP = mybir.ActivationFunctionType.Exp

        def s_pair(pt, i):
            qs = q2[:, pt * PT:(pt + 1) * PT]
            t = ps_s.tile([128, 2 * PT], f32, name=f"sp{pt}_{i}",
                          tag="spair")
            nc.tensor.matmul(t[:, 0:PT], k2[:, (2 * i) * QC:
                                             (2 * i + 1) * QC],
                             qs, start=True, stop=True)
            nc.tensor.matmul(t[:, PT:2 * PT], k2[:, (2 * i + 1) * QC:
                                                 (2 * i + 2) * QC],
                             qs, start=True, stop=True)
            return t

        def mk_exp(pt, i, t):
            e = exps_pool.tile([128, 2 * PT], bf16, name=f"e{pt}_{i}",
                               tag="exps")
            nc.scalar.activation(e, t, func=EXP)
            return e

        def preheat(pt):
            """Pair 0 of ptile pt via the scratch psum banks + pair 1's S,
            emitted while the previous ptile is still streaming, so the
            next ptile's Z matmuls can start with zero PE idle."""
            qs = q2[:, pt * PT:(pt + 1) * PT]
            sa = ps_w.tile([128, PT], f32, name=f"sa{pt}", tag="scr")
            nc.tensor.matmul(sa, k2[:, 0:QC], qs, start=True, stop=True)
            sb = ps_w.tile([128, PT], f32, name=f"sb{pt}", tag="scr")
            nc.tensor.matmul(sb, k2[:, QC:2 * QC], qs, start=True,
                             stop=True)
            s1 = s_pair(pt, 1)
            e0 = exps_pool.tile([128, 2 * PT], bf16, name=f"e{pt}_0",
                                tag="exps")
            nc.scalar.activation(e0[:, 0:PT], sa, func=EXP)
            nc.scalar.activation(e0[:, PT:2 * PT], sb, func=EXP)
            return e0, s1

        def emit_sums(pt, accf, saved_e):
            sums = ps_w.tile([128, PT], f32, name=f"sums{pt}", tag="scr")
            nc.tensor.matmul(sums, ones_b, accf, start=True, stop=False)
            for idx, j in enumerate(PE_PAIRS):
                e = saved_e[j]
                nc.tensor.matmul(sums, ones_b, e[:, 0:PT],
                                 start=False, stop=False)
                nc.tensor.matmul(sums, ones_b, e[:, PT:2 * PT],
                                 start=False, stop=(idx == len(PE_PAIRS) - 1))
            return sums

        tail_finish = None
        proj_kq(0)
        ph = preheat(0)
        proj_v(0)
        for pt in range(NPT):
            last = pt == NPT - 1
            pz0 = ps_z.tile([128, PT], f32, name=f"pz0_{pt}", tag="pz0")
            pz1 = ps_z.tile([128, PT], f32, name=f"pz1_{pt}", tag="pz1")
            acc_a = sum_pool.tile([128, 2 * PT], f32, name=f"acca{pt}",
                                  tag="acca")
            acc_b = sum_pool.tile([128, 2 * PT], f32, name=f"accb{pt}",
                                  tag="accb")
            acc_p = sum_pool.tile([128, 2 * PT], f32, name=f"accp{pt}",
                                  tag="accp")
            saved_e = {}
            ebuf = {}
            sums_ps = None

            pend = [ph[1]]
            E = {0: ph[0]}

            for i in range(PAIRS):
                if pt == 0 and i in (2, 5, 9):
                    proj_chunk({2: 1, 5: 2, 9: 3}[i])
                if i + 2 < PAIRS:
                    pend.append(s_pair(pt, i + 2))
                if i + 1 < PAIRS:
                    E[i + 1] = mk_exp(pt, i + 1, pend.pop(0))
                if i == 0 and tail_finish is not None:
                    tail_finish[0]()  # evict prev pz before Z reuses banks
                if i == 2 and tail_finish is not None:
                    tail_finish[1]()  # prev denominator + normalize + store
                    tail_finish = None
                e = E.pop(i)
                c0, c1 = 2 * i, 2 * i + 1
                e0, e1 = e[:, 0:PT], e[:, PT:2 * PT]
                nc.tensor.matmul(pz0, vt[c0][:, 0:128], e0,
                                 start=(i == 0), stop=False)
                nc.tensor.matmul(pz0, vt[c1][:, 0:128], e1,
                                 start=False, stop=(i == PAIRS - 1))
                nc.tensor.matmul(pz1, vt[c0][:, 128:256], e0,
                                 start=(i == 0), stop=False)
                nc.tensor.matmul(pz1, vt[c1][:, 128:256], e1,
                                 start=False, stop=(i == PAIRS - 1))
                ebuf[i] = e
                if (not last) and i in PE_PAIRS:
                    saved_e[i] = e       # summed on PE after the last Z
                if last and i in PE_PAIRS_LAST:
                    # interleave denominator matmuls with the Z stream
                    if sums_ps is None:
                        sums_ps = ps_w.tile([128, PT], f32,
                                            name=f"sums{pt}", tag="scr")
                        nc.tensor.matmul(sums_ps, ones_b, e0,
                                         start=True, stop=False)
                    else:
                        nc.tensor.matmul(sums_ps, ones_b, e0,
                                         start=False, stop=False)
                    nc.tensor.matmul(sums_ps, ones_b, e1,
                                     start=False, stop=False)
                # Pool chain: pairs 1,3,5,7,9; two-operand first add
                if i == 3:
                    nc.gpsimd.tensor_add(acc_p, ebuf[1], ebuf[3])
                elif i in (5, 7, 9):
                    nc.gpsimd.tensor_add(acc_p, acc_p, ebuf[i])
                if i == 10:
                    # Pool folds its own accumulator while DVE still adds
                    acc_pr = sum_pool.tile([128, PT], f32, name=f"apr{pt}",
                                           tag="accpr")
                    nc.gpsimd.tensor_add(acc_pr, acc_p[:, 0:PT],
                                         acc_p[:, PT:2 * PT])
                # DVE chains: non-last a={0,4,8,11} b={2,6,10};
                #             last     a={0,4,8}    b={2,6,10}
                if i == 4:
                    nc.vector.tensor_add(acc_a, ebuf[0], ebuf[4])
                elif i == 6:
                    nc.vector.tensor_add(acc_b, ebuf[2], ebuf[6])
                elif i == 8:
                    nc.vector.tensor_add(acc_a, acc_a, ebuf[8])
                elif i == 10:
                    nc.vector.tensor_add(acc_b, acc_b, ebuf[10])
                elif i == 11 and not last:
                    nc.vector.tensor_add(acc_a, acc_a, ebuf[11])
                if i == (10 if last else 11):
                    # fold-first merge: four short [128, PT] adds
                    acc_ua = sum_pool.tile([128, PT], f32, name=f"ua{pt}",
                                           tag="accua")
                    nc.vector.tensor_add(acc_ua, acc_a[:, 0:PT],
                                         acc_a[:, PT:2 * PT])
                    acc_ub = sum_pool.tile([128, PT], f32, name=f"ub{pt}",
                                           tag="accub")
                    nc.vector.tensor_add(acc_ub, acc_b[:, 0:PT],
                                         acc_b[:, PT:2 * PT])
                    acc_u = sum_pool.tile([128, PT], f32, name=f"au{pt}",
                                          tag="accu")
                    nc.vector.tensor_add(acc_u, acc_ua, acc_ub)
                if i == 11:
                    accf = sum_pool.tile([128, PT], bf16, name=f"af{pt}",
                                         tag="accf")
                    nc.vector.tensor_add(accf, acc_u, acc_pr)
                if i == 14 and pt + 1 < NPT:
                    ph = preheat(pt + 1)

            if last:
                # accf matmul last (chains have finished by now), then the
                # short normalize+store tail straight out of PSUM
                nc.tensor.matmul(sums_ps, ones_b, accf,
                                 start=False, stop=True)
                bcast = sum_pool.tile([128, PT], f32, name=f"bc{pt}",
                                      tag="bcast")
                nc.vector.reciprocal_approx_fast(out=bcast, in_=sums_ps)
                out0 = out_pool.tile([128, PT], fp16, name=f"o0_{pt}",
                                     tag="out0")
                out1 = out_pool.tile([128, PT], fp16, name=f"o1_{pt}",
                                     tag="out1")
                nc.vector.tensor_mul(out0, pz0, bcast)
                nc.vector.tensor_mul(out1, pz1, bcast)
                base = pt * PT
                hp = PT // 2
                nc.sync.dma_start(out=zout[0:128, base:base + PT],
                                  in_=out0)
                nc.scalar.dma_start(out=zout[128:256, base:base + hp],
                                    in_=out1[:, 0:hp])
                nc.gpsimd.dma_start(out=zout[128:256, base + hp:base + PT],
                                    in_=out1[:, hp:PT])
            else:
                # ---- deferred tail: evict pz, then denominator+normalize
                def make_tail(pt=pt, pz0=pz0, pz1=pz1, accf=accf,
                              saved_e=dict(saved_e)):
                    st = {}

                    def evict():
                        st["zr0"] = out_pool.tile([128, PT], f32,
                                                  name=f"zr0_{pt}",
                                                  tag="zr0")
                        st["zr1"] = out_pool.tile([128, PT], f32,
                                                  name=f"zr1_{pt}",
                                                  tag="zr1")
                        nc.vector.tensor_copy(st["zr0"], pz0)
                        nc.vector.tensor_copy(st["zr1"], pz1)

                    def finish():
                        sums = emit_sums(pt, accf, saved_e)
                        bcast = sum_pool.tile([128, PT], f32, name=f"bc{pt}",
                                              tag="bcast")
                        nc.vector.reciprocal_approx_fast(out=bcast, in_=sums)
                        out0 = out_pool.tile([128, PT], fp16,
                                             name=f"o0_{pt}", tag="out0")
                        out1 = out_pool.tile([128, PT], fp16,
                                             name=f"o1_{pt}", tag="out1")
                        nc.vector.tensor_mul(out0, st["zr0"], bcast)
                        nc.vector.tensor_mul(out1, st["zr1"], bcast)
                        nc.sync.dma_start(
                            out=zout[0:128, pt * PT:(pt + 1) * PT], in_=out0)
                        nc.sync.dma_start(
                            out=zout[128:256, pt * PT:(pt + 1) * PT],
                            in_=out1)
                    return evict, finish

                tail_finish = make_tail()

    nc.compile()
    _cache["nc"] = nc
    return nc


def _in_maps(x, q_w, k_w, v_w):
    xf = np.ascontiguousarray(x.reshape(B, DIN, HW), dtype=np.float32)
    qwT = np.asarray(q_w, np.float32).T            # [din, dk]
    # k_w halved: k2 holds K/2 on both partition halves, S contracts K=128
    kwT = np.asarray(k_w, np.float32).T * 0.5
    vwT = np.asarray(v_w, np.float32).T            # [din, dv]
    # per-half embedded weights: [k_h | q_h | v_h] (384 cols)
    w0 = np.concatenate([kwT[0:128], qwT[0:128], vwT[0:128]], axis=1)
    w1 = np.concatenate([kwT[128:256], qwT[128:256], vwT[128:256]], axis=1)
    maps = []
    for c in range(N_CORES):
        b, half = divmod(c, 2)
        xbc = xf[b] if half == 0 else np.ascontiguousarray(
            np.roll(xf[b], -PQ, axis=1))
        xbig = np.concatenate(
            [np.concatenate([w0, xbc[0:128]], axis=1),
             np.concatenate([w1, xbc[128:256]], axis=1)],
            axis=0).astype(np.float16)
        maps.append({"xb": xbig})
    return maps


def _gather(results):
    z = np.empty((B, DV, HW), np.float32)
    for c in range(N_CORES):
        b, half = divmod(c, 2)
        z[b][:, half * PQ:(half + 1) * PQ] = \
            results[c]["zout"].astype(np.float32)
    return z.reshape(B, DV, H, W)


def _run(x, q_w, k_w, v_w, trace=False):
    from concourse import bass_utils
    nc = _build()
    res = bass_utils.run_bass_kernel_spmd(
        nc, _in_maps(x, q_w, k_w, v_w), core_ids=list(range(N_CORES)),
        trace=trace)
    return _gather(res.results), res


def kernel(x, q_w, k_w, v_w):
    z, _ = _run(x, q_w, k_w, v_w)
    return z
